# revision 13
# baseline (speedup 1.0000x reference)
"""Trainium2 Bass kernel for an MoE layer (8 experts, top-2 routing, SwiGLU
experts + dense shared expert).

Strategy (expert-parallel over 8 NeuronCores):
  - Router (gate matmul + softmax + top-k + combine weights) and the aux/z
    losses are computed on host with jax-on-CPU, replicating the reference
    math op-for-op so the token->expert assignment matches bit-exactly.
  - Each core c is assigned expert c: the tokens routed to expert c are
    gathered on host, padded to a fixed capacity CE=1024, transposed to
    feature-major layout, and shipped to core c together with that expert's
    weights (pre-blocked on host so every weight DMA is a contiguous
    8KB-per-partition read). The core computes the full SwiGLU with
    float32r matmuls. Tokens beyond capacity (rare, ~90 pairs for the
    fixed seed) are computed exactly on host.
  - The shared expert is data-parallel: core c computes the shared SwiGLU
    for tokens [512c, 512(c+1)).
  - Host applies the top-2 combine weights while scattering expert outputs
    back, and adds the shared output.
"""

import numpy as np

D = 2048          # model dim
I = 1024          # expert intermediate dim
E = 8             # experts == cores
TOPK = 2
NTOK = 4096       # B*T
CE = 1024         # per-expert token capacity (2 x 512)
CS = NTOK // 8    # shared-expert tokens per core
KD = D // 128     # 16 contraction tiles over D
KI = I // 128     # 8 contraction tiles over I
MI = I // 128     # 8 stage-1 output tiles
MD = D // 128     # 16 stage-2 output tiles
NE_T = 512        # moving-dim tile, expert phase (CE = 2*512)
NS_T = 512        # moving-dim tile, shared phase

AUX_COEFF = 0.01
Z_COEFF = 0.001

_PROGRAM = None


def _emit(tc, mybir, aps):
    nc = tc.nc
    f32r, f32 = mybir.dt.float32r, mybir.dt.float32
    SILU = mybir.ActivationFunctionType.Silu
    NE = CE // NE_T   # 2 moving tiles in expert phase

    xe_r = aps["xeT"].rearrange("(ko p) c -> p ko c", p=128)    # [128,16,CE]
    xs_r = aps["xsT"].rearrange("(ko p) c -> p ko c", p=128)    # [128,16,CS]
    eo_r = aps["eoT"].rearrange("(mo p) c -> mo p c", p=128)    # [16,128,CE]
    sh_r = aps["shT"].rearrange("(mo p) c -> mo p c", p=128)
    w1_b, w3_b, w2_b = aps["w1B"], aps["w3B"], aps["w2B"]       # [m,128,ko,128]
    sw1_b, sw3_b, sw2_b = aps["sw1B"], aps["sw3B"], aps["sw2B"]

    with (tc.tile_pool(name="pw", bufs=6) as pw,
          tc.tile_pool(name="pw2", bufs=3) as pw2,
          tc.tile_pool(name="pt", bufs=4) as pt,
          tc.tile_pool(name="pout", bufs=3) as pout,
          tc.tile_pool(name="pd", bufs=1) as pd,
          tc.tile_pool(name="ps", bufs=8, space="PSUM") as ps):
        # PE warm-up: ~3.5us of dummy matmuls so HAM un-throttles while the
        # first real input DMAs are still in flight.
        bf16 = mybir.dt.bfloat16
        dw = pd.tile([128, 128], bf16, name="dw")
        nc.gpsimd.memset(dw[:], 0.0)
        dx = pd.tile([128, NE_T], bf16, name="dx")
        nc.gpsimd.memset(dx[:], 0.0)
        for _ in range(36):
            dps = ps.tile([128, NE_T], f32, tag="ps", name="dps")
            nc.tensor.matmul(dps[:], dw[:], dx[:], start=True, stop=True)
        with tc.tile_pool(name="ph", bufs=1) as ph:
            h = ph.tile([128, MI, CE], f32r, name="h")
            with tc.tile_pool(name="px", bufs=1) as px:
                xe = px.tile([128, KD, CE], f32r, name="xe")
                # m=0 weight columns first on each ring, then the xe chunks:
                # ring order is emission order, and the first chains need
                # w1c[0] + xe chunk 0 as early as possible.
                w1c0 = pw.tile([128, KD, 128], f32r, tag="wc", name="w1c")
                nc.sync.dma_start(w1c0[:], w1_b[0])
                nc.scalar.dma_start(xe[:, 0:2, :], xe_r[:, 0:2, :])
                w3c0 = pw.tile([128, KD, 128], f32r, tag="wc", name="w3c")
                nc.scalar.dma_start(w3c0[:], w3_b[0])
                for q in range(1, 8):
                    eng = nc.sync if q % 2 == 1 else nc.scalar
                    eng.dma_start(xe[:, q * 2:(q + 1) * 2, :],
                                  xe_r[:, q * 2:(q + 1) * 2, :])
                # ---- Expert stage 1: h = silu(w1 @ xe) * (w3 @ xe) ----
                for m in range(MI):
                    if m == 0:
                        w1c, w3c = w1c0, w3c0
                    else:
                        w1c = pw.tile([128, KD, 128], f32r, tag="wc", name="w1c")
                        nc.sync.dma_start(w1c[:], w1_b[m])
                        w3c = pw.tile([128, KD, 128], f32r, tag="wc", name="w3c")
                        nc.scalar.dma_start(w3c[:], w3_b[m])
                    if m < 2:
                        # prefetch the first down-proj weight columns early,
                        # on the otherwise-idle gpsimd queue so stage 2's
                        # first matmuls never wait on the busy weight rings
                        w2p = pw2.tile([128, KI, 128], f32r, tag="w2c",
                                       name="w2p")
                        nc.gpsimd.dma_start(w2p[:], w2_b[m])
                        if m == 0:
                            w2pre = []
                        w2pre.append(w2p)
                    pgs = [ps.tile([128, NE_T], f32, tag="ps", name=f"pg{n}")
                           for n in range(NE)]
                    for ko in range(KD):
                        for n in range(NE):
                            nsl = slice(n * NE_T, (n + 1) * NE_T)
                            nc.tensor.matmul(pgs[n][:], w1c[:, ko, :],
                                             xe[:, ko, nsl],
                                             start=(ko == 0), stop=(ko == KD - 1))
                    pus = [ps.tile([128, NE_T], f32, tag="ps", name=f"pu{n}")
                           for n in range(NE)]
                    for ko in range(KD):
                        for n in range(NE):
                            nsl = slice(n * NE_T, (n + 1) * NE_T)
                            nc.tensor.matmul(pus[n][:], w3c[:, ko, :],
                                             xe[:, ko, nsl],
                                             start=(ko == 0), stop=(ko == KD - 1))
                    for n in range(NE):
                        nsl = slice(n * NE_T, (n + 1) * NE_T)
                        sg = pt.tile([128, NE_T], f32, tag="sg", name="sg")
                        nc.scalar.activation(sg[:], pgs[n][:], SILU)
                        nc.vector.tensor_mul(h[:, m, nsl], sg[:], pus[n][:])
            # px closed: xe space reusable
            with (tc.tile_pool(name="pxs", bufs=1) as pxs,
                  tc.tile_pool(name="phs", bufs=1) as phs):
                xs = pxs.tile([128, KD, CS], f32r, name="xs")
                hs = phs.tile([128, KI, CS], f32r, name="hs")
                # ---- Expert stage 2: eoT = w2 @ h ----
                for m in range(MD):
                    if m < 2:
                        w2c = w2pre[m]
                    else:
                        w2c = pw2.tile([128, KI, 128], f32r, tag="w2c",
                                       name="w2c")
                        eng = nc.sync if m % 2 == 0 else nc.scalar
                        eng.dma_start(w2c[:], w2_b[m])
                    if m == MD - 2:
                        # xs loads queue behind all E2 weight columns so they
                        # don't delay them; transfers overlap late E2 compute.
                        for q in range(4):
                            eng = nc.scalar if q % 2 == 0 else nc.sync
                            eng.dma_start(xs[:, q * 4:(q + 1) * 4, :],
                                          xs_r[:, q * 4:(q + 1) * 4, :])
                    ot = pout.tile([128, CE], f32, tag="ot", name="ot")
                    pos = [ps.tile([128, NE_T], f32, tag="ps", name=f"po{n}")
                           for n in range(NE)]
                    for ko in range(KI):
                        for n in range(NE):
                            nsl = slice(n * NE_T, (n + 1) * NE_T)
                            nc.tensor.matmul(pos[n][:], w2c[:, ko, :],
                                             h[:, ko, nsl],
                                             start=(ko == 0), stop=(ko == KI - 1))
                    for n in range(NE):
                        nsl = slice(n * NE_T, (n + 1) * NE_T)
                        nc.vector.tensor_copy(ot[:, nsl], pos[n][:])
                    nc.gpsimd.dma_start(eo_r[m], ot[:])
                # ---- Shared stage 1 ----
                for m in range(MI):
                    s1c = pw.tile([128, KD, 128], f32r, tag="wc", name="s1c")
                    nc.sync.dma_start(s1c[:], sw1_b[m])
                    s3c = pw.tile([128, KD, 128], f32r, tag="wc", name="s3c")
                    nc.scalar.dma_start(s3c[:], sw3_b[m])
                    pg = ps.tile([128, NS_T], f32, tag="ps", name="spg")
                    for ko in range(KD):
                        nc.tensor.matmul(pg[:], s1c[:, ko, :], xs[:, ko, :],
                                         start=(ko == 0), stop=(ko == KD - 1))
                    pu = ps.tile([128, NS_T], f32, tag="ps", name="spu")
                    for ko in range(KD):
                        nc.tensor.matmul(pu[:], s3c[:, ko, :], xs[:, ko, :],
                                         start=(ko == 0), stop=(ko == KD - 1))
                    sg = pt.tile([128, NS_T], f32, tag="sg", name="ssg")
                    nc.scalar.activation(sg[:], pg[:], SILU)
                    nc.vector.tensor_mul(hs[:, m, :], sg[:], pu[:])
                # ---- Shared stage 2 ----
                for m in range(MD):
                    s2c = pw2.tile([128, KI, 128], f32r, tag="w2c", name="s2c")
                    eng = nc.sync if m % 2 == 0 else nc.scalar
                    eng.dma_start(s2c[:], sw2_b[m])
                    ot = pout.tile([128, CS], f32, tag="ot", name="sot")
                    po = ps.tile([128, NS_T], f32, tag="ps", name="spo")
                    for ko in range(KI):
                        nc.tensor.matmul(po[:], s2c[:, ko, :], hs[:, ko, :],
                                         start=(ko == 0), stop=(ko == KI - 1))
                    nc.vector.tensor_copy(ot[:], po[:])
                    if m >= MD - 2:
                        # tail: the last outputs go on the now-idle HWDGE
                        # rings instead of the slower SWDGE queue
                        (nc.scalar if m % 2 else nc.sync).dma_start(
                            sh_r[m], ot[:])
                    else:
                        nc.gpsimd.dma_start(sh_r[m], ot[:])


def _build_program():
    import concourse.tile as tile
    from concourse import bacc, mybir

    f32r, f32 = mybir.dt.float32r, mybir.dt.float32
    nc = bacc.Bacc("TRN2", target_bir_lowering=False, debug=False, num_devices=E)
    aps = {}
    for name, shape, dt, kind in [
        ("xeT", [D, CE], f32r, "ExternalInput"),
        ("xsT", [D, CS], f32r, "ExternalInput"),
        ("w1B", [MI, 128, KD, 128], f32r, "ExternalInput"),
        ("w3B", [MI, 128, KD, 128], f32r, "ExternalInput"),
        ("w2B", [MD, 128, KI, 128], f32r, "ExternalInput"),
        ("sw1B", [MI, 128, KD, 128], f32r, "ExternalInput"),
        ("sw3B", [MI, 128, KD, 128], f32r, "ExternalInput"),
        ("sw2B", [MD, 128, KI, 128], f32r, "ExternalInput"),
        ("eoT", [D, CE], f32, "ExternalOutput"),
        ("shT", [D, CS], f32, "ExternalOutput"),
    ]:
        aps[name] = nc.dram_tensor(name, shape, dt, kind=kind).ap()

    with tile.TileContext(nc) as tc:
        _emit(tc, mybir, aps)
    nc.compile()
    return nc


def _block_up(wT):
    """[D(=ko*128+p), I(=m*128+i)] -> [m, p, ko, i] contiguous blocks."""
    return np.ascontiguousarray(
        wT.reshape(KD, 128, MI, 128).transpose(2, 1, 0, 3))


def _block_down(wT):
    """[I(=ko*128+p), D(=m*128+d)] -> [m, p, ko, d] contiguous blocks."""
    return np.ascontiguousarray(
        wT.reshape(KI, 128, MD, 128).transpose(2, 1, 0, 3))


def _router_host(xf, gate_w):
    """Replicate the reference router + losses with jax on CPU (bit-exact
    wrt the reference's fp32 op sequence)."""
    import jax
    import jax.numpy as jnp

    cpu = jax.devices("cpu")[0]
    with jax.default_device(cpu):
        xf_j = jnp.asarray(xf)
        gate_logits = xf_j @ jnp.asarray(gate_w).T
        scores = jax.nn.softmax(gate_logits, axis=-1)
        top_scores, top_idx = jax.lax.top_k(scores, TOPK)
        top_scores = top_scores / jnp.sum(top_scores, axis=-1, keepdims=True)
        one_hot = jax.nn.one_hot(top_idx, E, dtype=xf_j.dtype)
        combine = jnp.sum(one_hot * top_scores[..., None], axis=1)
        expert_mask = jnp.sum(one_hot, axis=1)
        f = jnp.mean(expert_mask, axis=0)
        p = jnp.mean(scores, axis=0)
        aux_loss = AUX_COEFF * E * jnp.sum(f * p)
        z = jax.nn.logsumexp(gate_logits.astype(jnp.float32), axis=-1)
        z_loss = Z_COEFF * jnp.mean(z ** 2)
        total_loss = aux_loss + z_loss
    return (np.asarray(top_idx), np.asarray(combine),
            np.asarray(total_loss, dtype=np.float32))


def _silu_np(x):
    return x / (1.0 + np.exp(-x))


def kernel(x, gate_w, w1, w3, w2, sw1, sw3, sw2):
    global _PROGRAM
    from concourse.bass_utils import run_bass_kernel_spmd

    x = np.ascontiguousarray(x, dtype=np.float32)
    gate_w = np.ascontiguousarray(gate_w, dtype=np.float32)
    w1 = np.ascontiguousarray(w1, dtype=np.float32)
    w3 = np.ascontiguousarray(w3, dtype=np.float32)
    w2 = np.ascontiguousarray(w2, dtype=np.float32)
    sw1 = np.ascontiguousarray(sw1, dtype=np.float32)
    sw3 = np.ascontiguousarray(sw3, dtype=np.float32)
    sw2 = np.ascontiguousarray(sw2, dtype=np.float32)

    B, T, Dm = x.shape
    xf = x.reshape(B * T, Dm)

    top_idx, combine, total_loss = _router_host(xf, gate_w)

    # Token dispatch; anything beyond capacity falls back to exact host math.
    idxs = []
    overflow = {}
    for e in range(E):
        idx = np.nonzero((top_idx == e).any(axis=1))[0]
        if len(idx) > CE:
            overflow[e] = idx[CE:]
            idx = idx[:CE]
        idxs.append(idx)

    sw1B = _block_up(sw1.T)
    sw3B = _block_up(sw3.T)
    sw2B = _block_down(sw2.T)
    in_maps = []
    for c in range(E):
        idx = idxs[c]
        xeT = np.zeros((D, CE), np.float32)
        xeT[:, :len(idx)] = xf[idx].T
        in_maps.append({
            "xeT": xeT,
            "xsT": np.ascontiguousarray(xf[c * CS:(c + 1) * CS].T),
            "w1B": _block_up(w1[c].T),
            "w3B": _block_up(w3[c].T),
            "w2B": _block_down(w2[c].T),
            "sw1B": sw1B,
            "sw3B": sw3B,
            "sw2B": sw2B,
        })

    if _PROGRAM is None:
        _PROGRAM = _build_program()
    res = run_bass_kernel_spmd(_PROGRAM, in_maps, core_ids=list(range(E)))

    out = np.empty((B * T, Dm), np.float32)
    for c in range(E):
        out[c * CS:(c + 1) * CS] = res.results[c]["shT"].T
    for e in range(E):
        idx = idxs[e]
        eo = res.results[e]["eoT"][:, :len(idx)].T
        out[idx] += combine[idx, e][:, None] * eo
    for e, extra in overflow.items():
        xo = xf[extra]                       # [n, D]
        g = xo @ w1[e].T
        u = xo @ w3[e].T
        eo = (_silu_np(g) * u) @ w2[e].T
        out[extra] += combine[extra, e][:, None] * eo

    return out.reshape(B, T, Dm), total_loss


# revision 19
# speedup vs baseline: 1.0477x; 1.0477x over previous
"""Trainium2 Bass kernel for an MoE layer (8 experts, top-2 routing, SwiGLU
experts + dense shared expert).

Strategy (expert-parallel over 8 NeuronCores):
  - Router (gate matmul + softmax + top-k + combine weights) and the aux/z
    losses are computed on host with jax-on-CPU, replicating the reference
    math op-for-op so the token->expert assignment matches bit-exactly.
  - Each core c is assigned expert c: the tokens routed to expert c are
    gathered on host, padded to a fixed capacity CE=1024, transposed to
    feature-major layout, and shipped to core c together with that expert's
    weights (pre-blocked on host so every weight DMA is a contiguous
    8KB-per-partition read). The core computes the full SwiGLU with
    float32r matmuls. Tokens beyond capacity (rare, ~90 pairs for the
    fixed seed) are computed exactly on host.
  - The shared expert is data-parallel: core c computes the shared SwiGLU
    for tokens [512c, 512(c+1)).
  - Host applies the top-2 combine weights while scattering expert outputs
    back, and adds the shared output.
"""

import numpy as np

D = 2048          # model dim
I = 1024          # expert intermediate dim
E = 8             # experts == cores
TOPK = 2
NTOK = 4096       # B*T
CE = 1024         # per-expert token capacity (2 x 512)
CS = NTOK // 8    # shared-expert tokens per core
KD = D // 128     # 16 contraction tiles over D
KI = I // 128     # 8 contraction tiles over I
MI = I // 128     # 8 stage-1 output tiles
MD = D // 128     # 16 stage-2 output tiles
NE_T = 512        # moving-dim tile, expert phase (CE = 2*512)
NS_T = 512        # moving-dim tile, shared phase

AUX_COEFF = 0.01
Z_COEFF = 0.001

_PROGRAM = None


def _emit(tc, mybir, aps):
    nc = tc.nc
    f32r, f32 = mybir.dt.float32r, mybir.dt.float32
    SILU = mybir.ActivationFunctionType.Silu
    NE = CE // NE_T   # 2 moving tiles in expert phase

    xe_r = aps["xeT"].rearrange("(ko p) c -> p ko c", p=128)    # [128,16,CE]
    xs_r = aps["xsT"].rearrange("(ko p) c -> p ko c", p=128)    # [128,16,CS]
    eo_r = aps["eoT"].rearrange("(mo p) c -> mo p c", p=128)    # [16,128,CE]
    sh_r = aps["shT"].rearrange("(mo p) c -> mo p c", p=128)
    w1_b, w3_b, w2_b = aps["w1B"], aps["w3B"], aps["w2B"]       # [m,128,ko,128]
    sw1_b, sw3_b, sw2_b = aps["sw1B"], aps["sw3B"], aps["sw2B"]

    with (tc.tile_pool(name="pw", bufs=6) as pw,
          tc.tile_pool(name="pw2", bufs=5) as pw2,
          tc.tile_pool(name="pt", bufs=4) as pt,
          tc.tile_pool(name="pout", bufs=3) as pout,
          tc.tile_pool(name="pd", bufs=1) as pd,
          tc.tile_pool(name="ps", bufs=8, space="PSUM") as ps):
        # PE warm-up: ~3.5us of dummy matmuls so HAM un-throttles while the
        # first real input DMAs are still in flight.
        bf16 = mybir.dt.bfloat16
        dw = pd.tile([128, 128], bf16, name="dw")
        nc.gpsimd.memset(dw[:], 0.0)
        dx = pd.tile([128, NE_T], bf16, name="dx")
        nc.gpsimd.memset(dx[:], 0.0)
        for _ in range(20):
            dps = ps.tile([128, NE_T], f32, tag="ps", name="dps")
            nc.tensor.matmul(dps[:], dw[:], dx[:], start=True, stop=True)
        with tc.tile_pool(name="ph", bufs=1) as ph:
            h = ph.tile([128, MI, CE], f32r, name="h")
            with tc.tile_pool(name="px", bufs=1) as px:
                xe = px.tile([128, KD, CE], f32r, name="xe")
                # xe streams alone on the scalar ring; all E1 weights stream
                # on the sync ring (E1 is the long phase with DMA slack —
                # keeping the two streams on separate rings avoids the
                # FIFO-ordering contention that delays the first chains).
                for q in range(8):
                    nc.scalar.dma_start(xe[:, q * 2:(q + 1) * 2, :],
                                        xe_r[:, q * 2:(q + 1) * 2, :])
                # ---- Expert stage 1: h = silu(w1 @ xe) * (w3 @ xe) ----
                for m in range(MI):
                    w1c = pw.tile([128, KD, 128], f32r, tag="wc", name="w1c")
                    nc.sync.dma_start(w1c[:], w1_b[m])
                    w3c = pw.tile([128, KD, 128], f32r, tag="wc", name="w3c")
                    nc.sync.dma_start(w3c[:], w3_b[m])
                    if m < 2:
                        # prefetch the first down-proj weight columns early,
                        # on the otherwise-idle gpsimd queue so stage 2's
                        # first matmuls never wait on the busy weight rings
                        w2p = pw2.tile([128, KI, 128], f32r, tag="w2c",
                                       name="w2p")
                        nc.gpsimd.dma_start(w2p[:], w2_b[m])
                        if m == 0:
                            w2pre = []
                        w2pre.append(w2p)
                    pgs = [ps.tile([128, NE_T], f32, tag="ps", name=f"pg{n}")
                           for n in range(NE)]
                    for ko in range(KD):
                        for n in range(NE):
                            nsl = slice(n * NE_T, (n + 1) * NE_T)
                            nc.tensor.matmul(pgs[n][:], w1c[:, ko, :],
                                             xe[:, ko, nsl],
                                             start=(ko == 0), stop=(ko == KD - 1))
                    pus = [ps.tile([128, NE_T], f32, tag="ps", name=f"pu{n}")
                           for n in range(NE)]
                    for ko in range(KD):
                        for n in range(NE):
                            nsl = slice(n * NE_T, (n + 1) * NE_T)
                            nc.tensor.matmul(pus[n][:], w3c[:, ko, :],
                                             xe[:, ko, nsl],
                                             start=(ko == 0), stop=(ko == KD - 1))
                    for n in range(NE):
                        nsl = slice(n * NE_T, (n + 1) * NE_T)
                        sg = pt.tile([128, NE_T], f32, tag="sg", name="sg")
                        nc.scalar.activation(sg[:], pgs[n][:], SILU)
                        nc.vector.tensor_mul(h[:, m, nsl], sg[:], pus[n][:])
                    if m == MI - 2:
                        # prefetch the first shared-expert up-proj columns on
                        # the idle gpsimd queue
                        s1c0 = pw.tile([128, KD, 128], f32r, tag="wc",
                                       name="s1c0")
                        nc.gpsimd.dma_start(s1c0[:], sw1_b[0])
                        s3c0 = pw.tile([128, KD, 128], f32r, tag="wc",
                                       name="s3c0")
                        nc.gpsimd.dma_start(s3c0[:], sw3_b[0])
            # px closed: xe space reusable
            with (tc.tile_pool(name="pxs", bufs=1) as pxs,
                  tc.tile_pool(name="phs", bufs=1) as phs):
                xs = pxs.tile([128, KD, CS], f32r, name="xs")
                # xs on the scalar ring, which is idle once xe is in
                for q in range(4):
                    nc.scalar.dma_start(xs[:, q * 4:(q + 1) * 4, :],
                                        xs_r[:, q * 4:(q + 1) * 4, :])
                hs = phs.tile([128, KI, CS], f32r, name="hs")
                # ---- Expert stage 2 interleaved with shared stage 1 ----
                # Interleaving spreads shared-expert weight traffic (16.8MB)
                # over the whole E2 window instead of cramming it into the
                # short standalone S1 phase (which would need >300GB/s).
                s2pre = []
                for m in range(MD):
                    if m < 2:
                        w2c = w2pre[m]
                    else:
                        w2c = pw2.tile([128, KI, 128], f32r, tag="w2c",
                                       name="w2c")
                        nc.sync.dma_start(w2c[:], w2_b[m])
                    ot = pout.tile([128, CE], f32, tag="ot", name="ot")
                    pos = [ps.tile([128, NE_T], f32, tag="ps", name=f"po{n}")
                           for n in range(NE)]
                    for ko in range(KI):
                        for n in range(NE):
                            nsl = slice(n * NE_T, (n + 1) * NE_T)
                            nc.tensor.matmul(pos[n][:], w2c[:, ko, :],
                                             h[:, ko, nsl],
                                             start=(ko == 0), stop=(ko == KI - 1))
                    for n in range(NE):
                        nsl = slice(n * NE_T, (n + 1) * NE_T)
                        nc.vector.tensor_copy(ot[:, nsl], pos[n][:])
                    nc.gpsimd.dma_start(eo_r[m], ot[:])
                    if m == 8:
                        # prefetch first shared down-proj columns
                        for j in range(2):
                            s2p = pw2.tile([128, KI, 128], f32r, tag="w2c",
                                           name="s2p")
                            nc.gpsimd.dma_start(s2p[:], sw2_b[j])
                            s2pre.append(s2p)
                    if m % 2 == 1:
                        # ---- one shared-stage-1 iteration ----
                        s = m // 2
                        if s == 0:
                            s1c, s3c = s1c0, s3c0
                        else:
                            s1c = pw.tile([128, KD, 128], f32r, tag="wc",
                                          name="s1c")
                            nc.sync.dma_start(s1c[:], sw1_b[s])
                            s3c = pw.tile([128, KD, 128], f32r, tag="wc",
                                          name="s3c")
                            nc.scalar.dma_start(s3c[:], sw3_b[s])
                        pg = ps.tile([128, NS_T], f32, tag="ps", name="spg")
                        for ko in range(KD):
                            nc.tensor.matmul(pg[:], s1c[:, ko, :], xs[:, ko, :],
                                             start=(ko == 0), stop=(ko == KD - 1))
                        pu = ps.tile([128, NS_T], f32, tag="ps", name="spu")
                        for ko in range(KD):
                            nc.tensor.matmul(pu[:], s3c[:, ko, :], xs[:, ko, :],
                                             start=(ko == 0), stop=(ko == KD - 1))
                        sg = pt.tile([128, NS_T], f32, tag="sg", name="ssg")
                        nc.scalar.activation(sg[:], pg[:], SILU)
                        nc.vector.tensor_mul(hs[:, s, :], sg[:], pu[:])
                # ---- Shared stage 2 ----
                for m in range(MD):
                    if m < 2:
                        s2c = s2pre[m]
                    else:
                        s2c = pw2.tile([128, KI, 128], f32r, tag="w2c",
                                       name="s2c")
                        eng = nc.sync if m % 2 == 0 else nc.scalar
                        eng.dma_start(s2c[:], sw2_b[m])
                    ot = pout.tile([128, CS], f32, tag="ot", name="sot")
                    po = ps.tile([128, NS_T], f32, tag="ps", name="spo")
                    for ko in range(KI):
                        nc.tensor.matmul(po[:], s2c[:, ko, :], hs[:, ko, :],
                                         start=(ko == 0), stop=(ko == KI - 1))
                    nc.vector.tensor_copy(ot[:], po[:])
                    if m >= MD - 2:
                        # tail: the last outputs go on the now-idle HWDGE
                        # rings instead of the slower SWDGE queue
                        (nc.scalar if m % 2 else nc.sync).dma_start(
                            sh_r[m], ot[:])
                    else:
                        nc.gpsimd.dma_start(sh_r[m], ot[:])


def _build_program():
    import concourse.tile as tile
    from concourse import bacc, mybir

    f32r, f32 = mybir.dt.float32r, mybir.dt.float32
    nc = bacc.Bacc("TRN2", target_bir_lowering=False, debug=False, num_devices=E)
    aps = {}
    for name, shape, dt, kind in [
        ("xeT", [D, CE], f32r, "ExternalInput"),
        ("xsT", [D, CS], f32r, "ExternalInput"),
        ("w1B", [MI, 128, KD, 128], f32r, "ExternalInput"),
        ("w3B", [MI, 128, KD, 128], f32r, "ExternalInput"),
        ("w2B", [MD, 128, KI, 128], f32r, "ExternalInput"),
        ("sw1B", [MI, 128, KD, 128], f32r, "ExternalInput"),
        ("sw3B", [MI, 128, KD, 128], f32r, "ExternalInput"),
        ("sw2B", [MD, 128, KI, 128], f32r, "ExternalInput"),
        ("eoT", [D, CE], f32, "ExternalOutput"),
        ("shT", [D, CS], f32, "ExternalOutput"),
    ]:
        aps[name] = nc.dram_tensor(name, shape, dt, kind=kind).ap()

    with tile.TileContext(nc) as tc:
        _emit(tc, mybir, aps)
    nc.compile()
    return nc


def _block_up(wT):
    """[D(=ko*128+p), I(=m*128+i)] -> [m, p, ko, i] contiguous blocks."""
    return np.ascontiguousarray(
        wT.reshape(KD, 128, MI, 128).transpose(2, 1, 0, 3))


def _block_down(wT):
    """[I(=ko*128+p), D(=m*128+d)] -> [m, p, ko, d] contiguous blocks."""
    return np.ascontiguousarray(
        wT.reshape(KI, 128, MD, 128).transpose(2, 1, 0, 3))


def _router_host(xf, gate_w):
    """Replicate the reference router + losses with jax on CPU (bit-exact
    wrt the reference's fp32 op sequence)."""
    import jax
    import jax.numpy as jnp

    cpu = jax.devices("cpu")[0]
    with jax.default_device(cpu):
        xf_j = jnp.asarray(xf)
        gate_logits = xf_j @ jnp.asarray(gate_w).T
        scores = jax.nn.softmax(gate_logits, axis=-1)
        top_scores, top_idx = jax.lax.top_k(scores, TOPK)
        top_scores = top_scores / jnp.sum(top_scores, axis=-1, keepdims=True)
        one_hot = jax.nn.one_hot(top_idx, E, dtype=xf_j.dtype)
        combine = jnp.sum(one_hot * top_scores[..., None], axis=1)
        expert_mask = jnp.sum(one_hot, axis=1)
        f = jnp.mean(expert_mask, axis=0)
        p = jnp.mean(scores, axis=0)
        aux_loss = AUX_COEFF * E * jnp.sum(f * p)
        z = jax.nn.logsumexp(gate_logits.astype(jnp.float32), axis=-1)
        z_loss = Z_COEFF * jnp.mean(z ** 2)
        total_loss = aux_loss + z_loss
    return (np.asarray(top_idx), np.asarray(combine),
            np.asarray(total_loss, dtype=np.float32))


def _silu_np(x):
    return x / (1.0 + np.exp(-x))


def kernel(x, gate_w, w1, w3, w2, sw1, sw3, sw2):
    global _PROGRAM
    from concourse.bass_utils import run_bass_kernel_spmd

    x = np.ascontiguousarray(x, dtype=np.float32)
    gate_w = np.ascontiguousarray(gate_w, dtype=np.float32)
    w1 = np.ascontiguousarray(w1, dtype=np.float32)
    w3 = np.ascontiguousarray(w3, dtype=np.float32)
    w2 = np.ascontiguousarray(w2, dtype=np.float32)
    sw1 = np.ascontiguousarray(sw1, dtype=np.float32)
    sw3 = np.ascontiguousarray(sw3, dtype=np.float32)
    sw2 = np.ascontiguousarray(sw2, dtype=np.float32)

    B, T, Dm = x.shape
    xf = x.reshape(B * T, Dm)

    top_idx, combine, total_loss = _router_host(xf, gate_w)

    # Token dispatch; anything beyond capacity falls back to exact host math.
    idxs = []
    overflow = {}
    for e in range(E):
        idx = np.nonzero((top_idx == e).any(axis=1))[0]
        if len(idx) > CE:
            overflow[e] = idx[CE:]
            idx = idx[:CE]
        idxs.append(idx)

    sw1B = _block_up(sw1.T)
    sw3B = _block_up(sw3.T)
    sw2B = _block_down(sw2.T)
    in_maps = []
    for c in range(E):
        idx = idxs[c]
        xeT = np.zeros((D, CE), np.float32)
        xeT[:, :len(idx)] = xf[idx].T
        in_maps.append({
            "xeT": xeT,
            "xsT": np.ascontiguousarray(xf[c * CS:(c + 1) * CS].T),
            "w1B": _block_up(w1[c].T),
            "w3B": _block_up(w3[c].T),
            "w2B": _block_down(w2[c].T),
            "sw1B": sw1B,
            "sw3B": sw3B,
            "sw2B": sw2B,
        })

    if _PROGRAM is None:
        _PROGRAM = _build_program()
    res = run_bass_kernel_spmd(_PROGRAM, in_maps, core_ids=list(range(E)))

    out = np.empty((B * T, Dm), np.float32)
    for c in range(E):
        out[c * CS:(c + 1) * CS] = res.results[c]["shT"].T
    for e in range(E):
        idx = idxs[e]
        eo = res.results[e]["eoT"][:, :len(idx)].T
        out[idx] += combine[idx, e][:, None] * eo
    for e, extra in overflow.items():
        xo = xf[extra]                       # [n, D]
        g = xo @ w1[e].T
        u = xo @ w3[e].T
        eo = (_silu_np(g) * u) @ w2[e].T
        out[extra] += combine[extra, e][:, None] * eo

    return out.reshape(B, T, Dm), total_loss


# revision 21
# speedup vs baseline: 1.0980x; 1.0479x over previous
"""Trainium2 Bass kernel for an MoE layer (8 experts, top-2 routing, SwiGLU
experts + dense shared expert).

Strategy (expert-parallel over 8 NeuronCores):
  - Router (gate matmul + softmax + top-k + combine weights) and the aux/z
    losses are computed on host with jax-on-CPU, replicating the reference
    math op-for-op so the token->expert assignment matches bit-exactly.
  - Each core c is assigned expert c: the tokens routed to expert c are
    gathered on host, padded to a fixed capacity CE=1024, transposed to
    feature-major layout, and shipped to core c together with that expert's
    weights (pre-blocked on host so every weight DMA is a contiguous
    8KB-per-partition read). The core computes the full SwiGLU with
    float32r matmuls. Tokens beyond capacity (rare, ~90 pairs for the
    fixed seed) are computed exactly on host.
  - The shared expert is data-parallel: core c computes the shared SwiGLU
    for tokens [512c, 512(c+1)).
  - Host applies the top-2 combine weights while scattering expert outputs
    back, and adds the shared output.
"""

import numpy as np

D = 2048          # model dim
I = 1024          # expert intermediate dim
E = 8             # experts == cores
TOPK = 2
NTOK = 4096       # B*T
CE = 1024         # per-expert token capacity (2 x 512)
CS = NTOK // 8    # shared-expert tokens per core
KD = D // 128     # 16 contraction tiles over D
KI = I // 128     # 8 contraction tiles over I
MI = I // 128     # 8 stage-1 output tiles
MD = D // 128     # 16 stage-2 output tiles
NE_T = 512        # moving-dim tile, expert phase (CE = 2*512)
NS_T = 512        # moving-dim tile, shared phase

AUX_COEFF = 0.01
Z_COEFF = 0.001

_PROGRAM = None


def _emit(tc, mybir, aps):
    nc = tc.nc
    f32r, f32 = mybir.dt.float32r, mybir.dt.float32
    SILU = mybir.ActivationFunctionType.Silu
    NE = CE // NE_T   # 2 moving tiles in expert phase

    xe_r = aps["xeT"].rearrange("(ko p) c -> p ko c", p=128)    # [128,16,CE]
    xs_r = aps["xsT"].rearrange("(ko p) c -> p ko c", p=128)    # [128,16,CS]
    eo_r = aps["eoT"].rearrange("(mo p) c -> mo p c", p=128)    # [16,128,CE]
    sh_r = aps["shT"].rearrange("(mo p) c -> mo p c", p=128)
    w1_b, w3_b, w2_b = aps["w1B"], aps["w3B"], aps["w2B"]       # [m,128,ko,128]
    sw1_b, sw3_b, sw2_b = aps["sw1B"], aps["sw3B"], aps["sw2B"]

    with (tc.tile_pool(name="pw", bufs=6) as pw,
          tc.tile_pool(name="pw2", bufs=5) as pw2,
          tc.tile_pool(name="pt", bufs=4) as pt,
          tc.tile_pool(name="pout", bufs=3) as pout,
          tc.tile_pool(name="pd", bufs=1) as pd,
          tc.tile_pool(name="ps", bufs=8, space="PSUM") as ps):
        # PE warm-up: ~3.5us of dummy matmuls so HAM un-throttles while the
        # first real input DMAs are still in flight.
        bf16 = mybir.dt.bfloat16
        dw = pd.tile([128, 128], bf16, name="dw")
        nc.gpsimd.memset(dw[:], 0.0)
        dx = pd.tile([128, NE_T], bf16, name="dx")
        nc.gpsimd.memset(dx[:], 0.0)
        for _ in range(20):
            dps = ps.tile([128, NE_T], f32, tag="ps", name="dps")
            nc.tensor.matmul(dps[:], dw[:], dx[:], start=True, stop=True)
        with tc.tile_pool(name="ph", bufs=1) as ph:
            h = ph.tile([128, MI, CE], f32r, name="h")
            with tc.tile_pool(name="px", bufs=1) as px:
                xe = px.tile([128, KD, CE], f32r, name="xe")
                # Ring layout for the E1 ramp: the m=0 weight columns go at
                # the head of the sync ring, then xe streams down BOTH rings
                # (even chunks scalar, odd chunks sync) so the whole xe
                # arrives in half the time; remaining w1 columns follow on
                # sync, remaining w3 columns on scalar.
                w1c0 = pw.tile([128, KD, 128], f32r, tag="wc", name="w1c")
                nc.sync.dma_start(w1c0[:], w1_b[0])
                w3c0 = pw.tile([128, KD, 128], f32r, tag="wc", name="w3c")
                nc.sync.dma_start(w3c0[:], w3_b[0])
                for q in range(8):
                    eng = nc.scalar if q % 2 == 0 else nc.sync
                    eng.dma_start(xe[:, q * 2:(q + 1) * 2, :],
                                  xe_r[:, q * 2:(q + 1) * 2, :])
                # ---- Expert stage 1: h = silu(w1 @ xe) * (w3 @ xe) ----
                for m in range(MI):
                    if m == 0:
                        w1c, w3c = w1c0, w3c0
                    else:
                        w1c = pw.tile([128, KD, 128], f32r, tag="wc", name="w1c")
                        nc.sync.dma_start(w1c[:], w1_b[m])
                        w3c = pw.tile([128, KD, 128], f32r, tag="wc", name="w3c")
                        nc.scalar.dma_start(w3c[:], w3_b[m])
                    if m < 2:
                        # prefetch the first down-proj weight columns early,
                        # on the otherwise-idle gpsimd queue so stage 2's
                        # first matmuls never wait on the busy weight rings
                        w2p = pw2.tile([128, KI, 128], f32r, tag="w2c",
                                       name="w2p")
                        nc.gpsimd.dma_start(w2p[:], w2_b[m])
                        if m == 0:
                            w2pre = []
                        w2pre.append(w2p)
                    pgs = [ps.tile([128, NE_T], f32, tag="ps", name=f"pg{n}")
                           for n in range(NE)]
                    for ko in range(KD):
                        for n in range(NE):
                            nsl = slice(n * NE_T, (n + 1) * NE_T)
                            nc.tensor.matmul(pgs[n][:], w1c[:, ko, :],
                                             xe[:, ko, nsl],
                                             start=(ko == 0), stop=(ko == KD - 1))
                    pus = [ps.tile([128, NE_T], f32, tag="ps", name=f"pu{n}")
                           for n in range(NE)]
                    for ko in range(KD):
                        for n in range(NE):
                            nsl = slice(n * NE_T, (n + 1) * NE_T)
                            nc.tensor.matmul(pus[n][:], w3c[:, ko, :],
                                             xe[:, ko, nsl],
                                             start=(ko == 0), stop=(ko == KD - 1))
                    for n in range(NE):
                        nsl = slice(n * NE_T, (n + 1) * NE_T)
                        sg = pt.tile([128, NE_T], f32, tag="sg", name="sg")
                        nc.scalar.activation(sg[:], pgs[n][:], SILU)
                        nc.vector.tensor_mul(h[:, m, nsl], sg[:], pus[n][:])
                    if m == MI - 2:
                        # prefetch the first shared-expert up-proj columns on
                        # the idle gpsimd queue
                        s1c0 = pw.tile([128, KD, 128], f32r, tag="wc",
                                       name="s1c0")
                        nc.gpsimd.dma_start(s1c0[:], sw1_b[0])
                        s3c0 = pw.tile([128, KD, 128], f32r, tag="wc",
                                       name="s3c0")
                        nc.gpsimd.dma_start(s3c0[:], sw3_b[0])
            # px closed: xe space reusable
            with (tc.tile_pool(name="pxs", bufs=1) as pxs,
                  tc.tile_pool(name="phs", bufs=1) as phs):
                xs = pxs.tile([128, KD, CS], f32r, name="xs")
                # xs on the scalar ring, which is idle once xe is in
                for q in range(4):
                    nc.scalar.dma_start(xs[:, q * 4:(q + 1) * 4, :],
                                        xs_r[:, q * 4:(q + 1) * 4, :])
                hs = phs.tile([128, KI, CS], f32r, name="hs")
                # ---- Expert stage 2 interleaved with shared stage 1 ----
                # Interleaving spreads shared-expert weight traffic (16.8MB)
                # over the whole E2 window instead of cramming it into the
                # short standalone S1 phase (which would need >300GB/s).
                s2pre = []
                for m in range(MD):
                    if m < 2:
                        w2c = w2pre[m]
                    else:
                        w2c = pw2.tile([128, KI, 128], f32r, tag="w2c",
                                       name="w2c")
                        nc.sync.dma_start(w2c[:], w2_b[m])
                    ot = pout.tile([128, CE], f32, tag="ot", name="ot")
                    pos = [ps.tile([128, NE_T], f32, tag="ps", name=f"po{n}")
                           for n in range(NE)]
                    for ko in range(KI):
                        for n in range(NE):
                            nsl = slice(n * NE_T, (n + 1) * NE_T)
                            nc.tensor.matmul(pos[n][:], w2c[:, ko, :],
                                             h[:, ko, nsl],
                                             start=(ko == 0), stop=(ko == KI - 1))
                    for n in range(NE):
                        nsl = slice(n * NE_T, (n + 1) * NE_T)
                        nc.vector.tensor_copy(ot[:, nsl], pos[n][:])
                    nc.gpsimd.dma_start(eo_r[m], ot[:])
                    if m == 8:
                        # prefetch first shared down-proj columns
                        for j in range(2):
                            s2p = pw2.tile([128, KI, 128], f32r, tag="w2c",
                                           name="s2p")
                            nc.gpsimd.dma_start(s2p[:], sw2_b[j])
                            s2pre.append(s2p)
                    if 5 <= m <= 12:
                        # ---- one shared-stage-1 iteration ----
                        # (starting at m=5 gives the xs load, which must wait
                        # for E1 to release the xe SBUF region, time to land)
                        s = m - 5
                        if s == 0:
                            s1c, s3c = s1c0, s3c0
                        else:
                            s1c = pw.tile([128, KD, 128], f32r, tag="wc",
                                          name="s1c")
                            nc.sync.dma_start(s1c[:], sw1_b[s])
                            s3c = pw.tile([128, KD, 128], f32r, tag="wc",
                                          name="s3c")
                            nc.scalar.dma_start(s3c[:], sw3_b[s])
                        pg = ps.tile([128, NS_T], f32, tag="ps", name="spg")
                        for ko in range(KD):
                            nc.tensor.matmul(pg[:], s1c[:, ko, :], xs[:, ko, :],
                                             start=(ko == 0), stop=(ko == KD - 1))
                        pu = ps.tile([128, NS_T], f32, tag="ps", name="spu")
                        for ko in range(KD):
                            nc.tensor.matmul(pu[:], s3c[:, ko, :], xs[:, ko, :],
                                             start=(ko == 0), stop=(ko == KD - 1))
                        sg = pt.tile([128, NS_T], f32, tag="sg", name="ssg")
                        nc.scalar.activation(sg[:], pg[:], SILU)
                        nc.vector.tensor_mul(hs[:, s, :], sg[:], pu[:])
                # ---- Shared stage 2 ----
                for m in range(MD):
                    if m < 2:
                        s2c = s2pre[m]
                    else:
                        s2c = pw2.tile([128, KI, 128], f32r, tag="w2c",
                                       name="s2c")
                        eng = nc.sync if m % 2 == 0 else nc.scalar
                        eng.dma_start(s2c[:], sw2_b[m])
                    ot = pout.tile([128, CS], f32, tag="ot", name="sot")
                    po = ps.tile([128, NS_T], f32, tag="ps", name="spo")
                    for ko in range(KI):
                        nc.tensor.matmul(po[:], s2c[:, ko, :], hs[:, ko, :],
                                         start=(ko == 0), stop=(ko == KI - 1))
                    nc.vector.tensor_copy(ot[:], po[:])
                    if m >= MD - 2:
                        # tail: the last outputs go on the now-idle HWDGE
                        # rings instead of the slower SWDGE queue
                        (nc.scalar if m % 2 else nc.sync).dma_start(
                            sh_r[m], ot[:])
                    else:
                        nc.gpsimd.dma_start(sh_r[m], ot[:])


def _build_program():
    import concourse.tile as tile
    from concourse import bacc, mybir

    f32r, f32 = mybir.dt.float32r, mybir.dt.float32
    nc = bacc.Bacc("TRN2", target_bir_lowering=False, debug=False, num_devices=E)
    aps = {}
    for name, shape, dt, kind in [
        ("xeT", [D, CE], f32r, "ExternalInput"),
        ("xsT", [D, CS], f32r, "ExternalInput"),
        ("w1B", [MI, 128, KD, 128], f32r, "ExternalInput"),
        ("w3B", [MI, 128, KD, 128], f32r, "ExternalInput"),
        ("w2B", [MD, 128, KI, 128], f32r, "ExternalInput"),
        ("sw1B", [MI, 128, KD, 128], f32r, "ExternalInput"),
        ("sw3B", [MI, 128, KD, 128], f32r, "ExternalInput"),
        ("sw2B", [MD, 128, KI, 128], f32r, "ExternalInput"),
        ("eoT", [D, CE], f32, "ExternalOutput"),
        ("shT", [D, CS], f32, "ExternalOutput"),
    ]:
        aps[name] = nc.dram_tensor(name, shape, dt, kind=kind).ap()

    with tile.TileContext(nc) as tc:
        _emit(tc, mybir, aps)
    nc.compile()
    return nc


def _block_up(wT):
    """[D(=ko*128+p), I(=m*128+i)] -> [m, p, ko, i] contiguous blocks."""
    return np.ascontiguousarray(
        wT.reshape(KD, 128, MI, 128).transpose(2, 1, 0, 3))


def _block_down(wT):
    """[I(=ko*128+p), D(=m*128+d)] -> [m, p, ko, d] contiguous blocks."""
    return np.ascontiguousarray(
        wT.reshape(KI, 128, MD, 128).transpose(2, 1, 0, 3))


def _router_host(xf, gate_w):
    """Replicate the reference router + losses with jax on CPU (bit-exact
    wrt the reference's fp32 op sequence)."""
    import jax
    import jax.numpy as jnp

    cpu = jax.devices("cpu")[0]
    with jax.default_device(cpu):
        xf_j = jnp.asarray(xf)
        gate_logits = xf_j @ jnp.asarray(gate_w).T
        scores = jax.nn.softmax(gate_logits, axis=-1)
        top_scores, top_idx = jax.lax.top_k(scores, TOPK)
        top_scores = top_scores / jnp.sum(top_scores, axis=-1, keepdims=True)
        one_hot = jax.nn.one_hot(top_idx, E, dtype=xf_j.dtype)
        combine = jnp.sum(one_hot * top_scores[..., None], axis=1)
        expert_mask = jnp.sum(one_hot, axis=1)
        f = jnp.mean(expert_mask, axis=0)
        p = jnp.mean(scores, axis=0)
        aux_loss = AUX_COEFF * E * jnp.sum(f * p)
        z = jax.nn.logsumexp(gate_logits.astype(jnp.float32), axis=-1)
        z_loss = Z_COEFF * jnp.mean(z ** 2)
        total_loss = aux_loss + z_loss
    return (np.asarray(top_idx), np.asarray(combine),
            np.asarray(total_loss, dtype=np.float32))


def _silu_np(x):
    return x / (1.0 + np.exp(-x))


def kernel(x, gate_w, w1, w3, w2, sw1, sw3, sw2):
    global _PROGRAM
    from concourse.bass_utils import run_bass_kernel_spmd

    x = np.ascontiguousarray(x, dtype=np.float32)
    gate_w = np.ascontiguousarray(gate_w, dtype=np.float32)
    w1 = np.ascontiguousarray(w1, dtype=np.float32)
    w3 = np.ascontiguousarray(w3, dtype=np.float32)
    w2 = np.ascontiguousarray(w2, dtype=np.float32)
    sw1 = np.ascontiguousarray(sw1, dtype=np.float32)
    sw3 = np.ascontiguousarray(sw3, dtype=np.float32)
    sw2 = np.ascontiguousarray(sw2, dtype=np.float32)

    B, T, Dm = x.shape
    xf = x.reshape(B * T, Dm)

    top_idx, combine, total_loss = _router_host(xf, gate_w)

    # Token dispatch; anything beyond capacity falls back to exact host math.
    idxs = []
    overflow = {}
    for e in range(E):
        idx = np.nonzero((top_idx == e).any(axis=1))[0]
        if len(idx) > CE:
            overflow[e] = idx[CE:]
            idx = idx[:CE]
        idxs.append(idx)

    sw1B = _block_up(sw1.T)
    sw3B = _block_up(sw3.T)
    sw2B = _block_down(sw2.T)
    in_maps = []
    for c in range(E):
        idx = idxs[c]
        xeT = np.zeros((D, CE), np.float32)
        xeT[:, :len(idx)] = xf[idx].T
        in_maps.append({
            "xeT": xeT,
            "xsT": np.ascontiguousarray(xf[c * CS:(c + 1) * CS].T),
            "w1B": _block_up(w1[c].T),
            "w3B": _block_up(w3[c].T),
            "w2B": _block_down(w2[c].T),
            "sw1B": sw1B,
            "sw3B": sw3B,
            "sw2B": sw2B,
        })

    if _PROGRAM is None:
        _PROGRAM = _build_program()
    res = run_bass_kernel_spmd(_PROGRAM, in_maps, core_ids=list(range(E)))

    out = np.empty((B * T, Dm), np.float32)
    for c in range(E):
        out[c * CS:(c + 1) * CS] = res.results[c]["shT"].T
    for e in range(E):
        idx = idxs[e]
        eo = res.results[e]["eoT"][:, :len(idx)].T
        out[idx] += combine[idx, e][:, None] * eo
    for e, extra in overflow.items():
        xo = xf[extra]                       # [n, D]
        g = xo @ w1[e].T
        u = xo @ w3[e].T
        eo = (_silu_np(g) * u) @ w2[e].T
        out[extra] += combine[extra, e][:, None] * eo

    return out.reshape(B, T, Dm), total_loss
